# revision 5
# baseline (speedup 1.0000x reference)
"""Trainium2 Bass kernel for topk_masking (nn_CustomModule_8065948582484).

Reference semantics (per batch b):
  idx[b,f] = argmax(score[b,f,:196])                 (first index on ties)
  mask grows from a fixed prior region on a 14x14 grid; at frame f the
  argmax cell is added iff it is 4-adjacent to the current mask.
  out = [ones(B,1), masks frame-major] -> [B, 1+16*196] fp32.

Strategy (pure batch data-parallel across 8 cores, 2048 batches/core,
16 tiles of 128 batches on SBUF partitions):
  Phase A (per tile): frame-max via one DVE reduce. First-argmax via
    s = sign(m - sc) on ScalarE (exact 0 at the max, 1 below; Sign is
    the only activation func used anywhere -> no act-table reloads),
    u = s + p/1024 (one DVE bf16 2x add), then fold-min + min-reduce
    (DVE) -> idx/1024. Exact first-index tie semantics.
  Phase B (once per core, [128,16f,16t] domain): r via fused is_ge/add
    ladder, c/v exact int arithmetic, pitch-16 adjacency graph,
    sequential 16-step added-recurrence, idxd = added ? idx : -1.
  Phase C (per tile): fused one-hot+running-max chain, one
    scalar_tensor_tensor per frame (bf16 2x); ScalarE sign-converts
    bf16 masks -> fp32 out; DMA in/out overlapped.
"""

import sys

import numpy as np

for _p in ("/opt/trn_rl_repo",):
    if _p not in sys.path:
        sys.path.insert(0, _p)

from concourse import bacc, mybir, tile  # noqa: E402
from concourse.bass_utils import run_bass_kernel_spmd  # noqa: E402

B, F, P = 16384, 16, 196
N = 14  # grid side
NCORES = 8
BLOC = B // NCORES  # 2048
NT = BLOC // 128  # 16 tiles per core

ALU = mybir.AluOpType
AX = mybir.AxisListType
F32 = mybir.dt.float32
BF16 = mybir.dt.bfloat16
ACT = mybir.ActivationFunctionType

# frames whose sign-step runs on ScalarE (rest use a fused DVE stt)
KSIGN = 13


def build_nc():
    nc = bacc.Bacc(trn_type="TRN2", target_bir_lowering=False)
    score_d = nc.declare_dram_parameter("score", [BLOC, F, P], F32, isOutput=False)
    out_d = nc.declare_dram_parameter("out", [BLOC, 1 + F * P], F32, isOutput=True)

    with tile.TileContext(nc) as tc:
        with (
            tc.tile_pool(name="consts", bufs=1) as cpool,
            tc.tile_pool(name="big", bufs=2) as bpool,
            tc.tile_pool(name="ph", bufs=1) as ppool,
        ):
            # ---- constants ----
            iotap = cpool.tile([128, P], BF16, name="iotap")
            nc.gpsimd.iota(
                iotap[:],
                pattern=[[1, P]],
                base=0,
                channel_multiplier=0,
                allow_small_or_imprecise_dtypes=True,
            )
            # p/1024 (exact in bf16); tie-break term for the min-reduce
            iotaq = cpool.tile([128, P], BF16, name="iotaq")
            nc.vector.tensor_scalar(iotaq[:], iotap[:], 1.0 / 1024.0, None, ALU.mult)
            pm256 = cpool.tile([128, P], BF16, name="pm256")
            nc.vector.tensor_scalar(pm256[:], iotap[:], 256.0, None, ALU.subtract)
            prior = cpool.tile([128, P], BF16, name="prior")
            nc.vector.memset(prior[:], 0.0)
            priorv = prior.rearrange("q (r c) -> q r c", r=N)
            nc.vector.memset(priorv[:, 4:14, 2:12], 1.0)

            # ---- phase A per tile: first-argmax ----
            # J holds idx/1024 for ScalarE-route frames, idx-256 for
            # stt-route frames.
            J = ppool.tile([128, NT, F], F32, name="J")
            for t in range(NT):
                r0 = t * 128
                sc = bpool.tile([128, F, P], F32, tag="sc", name="sc", bufs=3)
                nc.sync.dma_start(
                    out=sc.rearrange("q f p -> q (f p)"), in_=score_d[r0 : r0 + 128]
                )
                m = bpool.tile([128, F], F32, tag="m", name="m", bufs=3)
                nc.vector.tensor_reduce(m[:], sc[:], axis=AX.X, op=ALU.max)
                u = bpool.tile([128, F, P], BF16, tag="u", name="u", bufs=3)
                if KSIGN > 0:
                    sp = bpool.tile(
                        [128, KSIGN, P], BF16, tag="sp", name="sp", bufs=3
                    )
                    for f in range(KSIGN):
                        # s = sign(m - sc): 1 below the max, 0 at it
                        nc.scalar.activation(
                            sp[:, f, :],
                            sc[:, f, :],
                            ACT.Sign,
                            bias=m[:, f : f + 1],
                            scale=-1.0,
                        )
                    # u = s + p/1024; min over p -> (first argmax)/1024
                    nc.vector.tensor_tensor(
                        u[:, 0:KSIGN, :],
                        sp[:],
                        iotaq.unsqueeze(1).broadcast_to([128, KSIGN, P]),
                        ALU.add,
                    )
                for f in range(KSIGN, F):
                    # u = (sc == m) * (p - 256); min over p -> idx - 256
                    nc.vector.scalar_tensor_tensor(
                        u[:, f, :],
                        sc[:, f, :],
                        m[:, f : f + 1],
                        pm256[:],
                        ALU.is_equal,
                        ALU.mult,
                    )
                # folded min-reduce: 196 -> 98 -> 49 -> [128, F]
                u2 = bpool.tile([128, F, 98], BF16, tag="u2", name="u2", bufs=3)
                nc.vector.tensor_tensor(
                    u2[:], u[:, :, 0:98], u[:, :, 98:196], ALU.min
                )
                nc.vector.tensor_reduce(J[:, t, :], u2[:], axis=AX.X, op=ALU.min)

            # ---- phase B once per core on [128, F, NT] ----
            idx = ppool.tile([128, NT, F], F32, name="idx")
            if KSIGN > 0:
                nc.vector.tensor_scalar(
                    idx[:, :, 0:KSIGN], J[:, :, 0:KSIGN], 1024.0, None, ALU.mult
                )
            if KSIGN < F:
                nc.vector.tensor_scalar(
                    idx[:, :, KSIGN:F], J[:, :, KSIGN:F], 256.0, None, ALU.add
                )
            rr = ppool.tile([128, NT, F], BF16, name="rr")
            nc.vector.memset(rr[:], 0.0)
            for k in range(1, N):
                nc.vector.scalar_tensor_tensor(
                    rr[:], idx[:], 14.0 * k - 0.5, rr[:], ALU.is_ge, ALU.add
                )
            rf = ppool.tile([128, NT, F], F32, name="rf")
            nc.vector.tensor_copy(rf[:], rr[:])
            cc = ppool.tile([128, NT, F], F32, name="cc")
            nc.vector.scalar_tensor_tensor(
                cc[:], rf[:], -14.0, idx[:], ALU.mult, ALU.add
            )
            vv = ppool.tile([128, NT, F], F32, name="vv")
            nc.vector.scalar_tensor_tensor(
                vv[:], rf[:], 2.0, idx[:], ALU.mult, ALU.add
            )
            vb = ppool.tile([128, NT, F], BF16, name="vb")
            nc.vector.tensor_copy(vb[:], vv[:])

            # adjacency gg[q,f,t,e]: |v_e - v_f| in {1,16} via dv^2 in {1,256}
            dv = ppool.tile([128, F, NT, F], BF16, name="dv")
            nc.vector.tensor_tensor(
                dv[:],
                vb.unsqueeze(1).broadcast_to([128, F, NT, F]),
                vb.rearrange("q t f -> q f t").unsqueeze(3).broadcast_to(
                    [128, F, NT, F]
                ),
                ALU.subtract,
            )
            sq = ppool.tile([128, F, NT, F], BF16, name="sq")
            nc.vector.tensor_tensor(sq[:], dv[:], dv[:], ALU.mult)
            g1 = ppool.tile([128, F, NT, F], BF16, name="g1")
            nc.vector.tensor_scalar(g1[:], sq[:], 1.0, None, ALU.is_equal)
            gg = ppool.tile([128, F, NT, F], BF16, name="gg")
            nc.vector.scalar_tensor_tensor(
                gg[:], sq[:], 256.0, g1[:], ALU.is_equal, ALU.add
            )

            # A = (r>=3 & 2<=c<=11) | (r>=4 & 1<=c<=12)
            u3c = ppool.tile([128, NT, F], BF16, name="u3c")
            nc.vector.tensor_scalar(u3c[:], rf[:], 2.5, None, ALU.is_ge)
            u4c = ppool.tile([128, NT, F], BF16, name="u4c")
            nc.vector.tensor_scalar(u4c[:], rf[:], 3.5, None, ALU.is_ge)
            cm2 = ppool.tile([128, NT, F], F32, name="cm2")
            nc.vector.tensor_scalar(cm2[:], cc[:], 2.0, None, ALU.subtract)
            q1 = ppool.tile([128, NT, F], F32, name="q1")
            nc.vector.scalar_tensor_tensor(
                q1[:], cc[:], -11.0, cm2[:], ALU.add, ALU.mult
            )
            b1 = ppool.tile([128, NT, F], BF16, name="b1")
            nc.vector.tensor_scalar(b1[:], q1[:], 0.0, None, ALU.is_le)
            cm1 = ppool.tile([128, NT, F], F32, name="cm1")
            nc.vector.tensor_scalar(cm1[:], cc[:], 1.0, None, ALU.subtract)
            q2 = ppool.tile([128, NT, F], F32, name="q2")
            nc.vector.scalar_tensor_tensor(
                q2[:], cc[:], -12.0, cm1[:], ALU.add, ALU.mult
            )
            b2 = ppool.tile([128, NT, F], BF16, name="b2")
            nc.vector.tensor_scalar(b2[:], q2[:], 0.0, None, ALU.is_le)
            t1 = ppool.tile([128, NT, F], BF16, name="t1")
            nc.vector.tensor_tensor(t1[:], u3c[:], b1[:], ALU.logical_and)
            t2 = ppool.tile([128, NT, F], BF16, name="t2")
            nc.vector.tensor_tensor(t2[:], u4c[:], b2[:], ALU.logical_and)
            aa = ppool.tile([128, NT, F], BF16, name="aa")
            nc.vector.tensor_tensor(aa[:], t1[:], t2[:], ALU.logical_or)

            # sequential added-recurrence over frames (t-major, e innermost)
            added = ppool.tile([128, NT, F], BF16, name="added")
            nc.vector.memset(added[:], 0.0)
            t16 = ppool.tile([128, NT, F], BF16, name="t16")
            mxf = ppool.tile([128, NT], F32, name="mxf")
            for f in range(F):
                nc.vector.tensor_tensor(t16[:], added[:], gg[:, f, :, :], ALU.mult)
                nc.vector.tensor_reduce(mxf[:], t16[:], axis=AX.X, op=ALU.max)
                nc.vector.tensor_tensor(
                    added[:, :, f], mxf[:], aa[:, :, f], ALU.max
                )

            # idxd = added ? idx : -1
            ip1 = ppool.tile([128, NT, F], F32, name="ip1")
            nc.vector.tensor_scalar(ip1[:], idx[:], 1.0, None, ALU.add)
            idxd = ppool.tile([128, NT, F], F32, name="idxd")
            nc.vector.tensor_tensor(idxd[:], ip1[:], added[:], ALU.mult)
            nc.vector.tensor_scalar(idxd[:], idxd[:], 1.0, None, ALU.subtract)

            # ---- phase C per tile: fused mask build / convert / store ----
            for t in range(NT):
                r0 = t * 128
                masks = bpool.tile([128, F, P], BF16, tag="masks", name="masks")
                for f in range(F):
                    prev = prior[:] if f == 0 else masks[:, f - 1, :]
                    nc.vector.scalar_tensor_tensor(
                        masks[:, f, :],
                        iotap[:],
                        idxd[:, t, f : f + 1],
                        prev,
                        ALU.is_equal,
                        ALU.max,
                    )
                out_t = bpool.tile([128, 1 + F * P], F32, tag="out", name="out_t")
                nc.gpsimd.memset(out_t[:, 0:1], 1.0)
                # masks are 0/1 -> sign() is an exact bf16->fp32 convert and
                # keeps ScalarE on a single activation table
                nc.scalar.activation(
                    out_t[:, 1 : 1 + F * P],
                    masks.rearrange("q f p -> q (f p)"),
                    ACT.Sign,
                )
                nc.sync.dma_start(out=out_d[r0 : r0 + 128, :], in_=out_t[:])

    nc.compile()
    return nc


_nc = None


def _get_nc():
    global _nc
    if _nc is None:
        _nc = build_nc()
    return _nc


def kernel(score, topn=196):
    score = np.ascontiguousarray(np.asarray(score, dtype=np.float32)).reshape(B, F, P)
    nc = _get_nc()
    in_maps = [{"score": score[i * BLOC : (i + 1) * BLOC]} for i in range(NCORES)]
    res = run_bass_kernel_spmd(nc, in_maps, list(range(NCORES)))
    out = np.concatenate([res.results[i]["out"] for i in range(NCORES)], axis=0)
    return out
